# revision 1
# baseline (speedup 1.0000x reference)
"""APPNP (MLP + K-step personalized-PageRank propagation) on 8 TRN2 NeuronCores.

Strategy
--------
* Nodes are sharded across the 8 cores (12500 + 44 dead pad rows each).
* norm = dinv[src]*dinv[dst] factorizes, so each hop is:
      agg = A^T u  with  u = dinv*h  (gather u[src] per edge + segment-sum)
      u' = (0.9*dinv^2)*agg + 0.1*dinv*h0
* Per hop: AllGather of per-core u shards -> full u [100352, 64] in each
  core's DRAM, then indexed-row DMA gathers (gpsimd indirect_dma_start,
  one 128-row descriptor batch per call) pull per-edge source rows into
  SBUF in a static ELL layout (dst-tile x neighbor-slot grid, zero-row
  pads), and strided DVE tensor_reduce does the segment-sum.
* The ELL grid: per-core dsts are degree-sorted into 98 tiles of 128;
  tile t gets J[t] = max in-degree columns. Gather call (t, j) fetches,
  for all 128 dsts p of tile t, the row of their j-th in-neighbor
  (int32 indices, per-call index column [128, 1]).
* All static structure is computed on the host from edge_index inside
  kernel(); the 8 cores run one SPMD graph with identical shapes; the
  per-core index / scale tensors are kernel inputs.
"""

import math
import sys
import numpy as np

try:  # concourse ships in the container; add its repo root if not on path
    import concourse  # noqa: F401
except ImportError:  # pragma: no cover
    for _p in ("/root/.axon_site/_ro/trn_rl_repo", "/opt/trn_rl_repo"):
        if _p not in sys.path:
            sys.path.insert(0, _p)
    import concourse  # noqa: F401

# ---------------------------------------------------------------- constants
K_HOPS = 10
ALPHA = 0.1
P = 128  # partitions


class Cfg:
    def __init__(self, N=100000, E=1600000, IN=256, HID=256, OUT=64, cores=8,
                 chunk_cols=48, max_run_tiles=16, hops=K_HOPS):
        self.N, self.E, self.IN, self.HID, self.OUT = N, E, IN, HID, OUT
        self.cores = cores
        self.per = N // cores                       # real nodes per core
        self.T = math.ceil(self.per / P)            # tiles per core
        if self.T * P == self.per:
            self.T += 1  # need >=1 dead (always-zero) row for gather pads
        self.NSH = self.T * P                       # padded rows per core
        assert self.OUT == 64, "gather row hard-wired to 256B"
        self.chunk_cols = chunk_cols                # gather cols per msg tile
        self.max_run_tiles = max_run_tiles          # cap tiles per reduce op
        self.hops = hops
        self.F = self.OUT


FULL = Cfg()


# ---------------------------------------------------------------- host prep
def preprocess(edge_index, dinv, cfg, seed=0):
    """Static schedule + per-core input arrays.

    sched: shape-defining info shared by all cores (ELL tile degrees J,
    chunk/run grouping).  percore: per-core index & scale tensors.
    """
    N, C, T, NSH, F = cfg.N, cfg.cores, cfg.T, cfg.NSH, cfg.F
    per = cfg.per

    src = np.asarray(edge_index[0], dtype=np.int64)
    dst = np.asarray(edge_index[1], dtype=np.int64)
    loops = np.arange(N, dtype=np.int64)
    src_all = np.concatenate([src, loops])
    dst_all = np.concatenate([dst, loops])
    deg = np.bincount(dst_all, minlength=N)

    # ---- node -> core assignment (random balanced)
    rng = np.random.default_rng(seed)
    perm = rng.permutation(N)
    owner = np.empty(N, dtype=np.int64)
    for c in range(C):
        owner[perm[c * per:(c + 1) * per]] = c

    # ---- per-core rank: in-degree descending -> tight per-tile max degree
    rank = np.empty(N, dtype=np.int64)
    for c in range(C):
        nodes = perm[c * per:(c + 1) * per]
        order = np.argsort(-deg[nodes], kind="stable")
        rank[nodes[order]] = np.arange(per)
    tile_of = rank // P
    pslot = rank % P

    # ---- uniform-across-cores tile max degree
    Jt = np.zeros(T, dtype=np.int64)
    np.maximum.at(Jt, tile_of, deg)
    Jt[:] = np.maximum(Jt, 1)

    col_base = np.zeros(T, dtype=np.int64)
    col_base[1:] = np.cumsum(Jt)[:-1]
    COLS = int(Jt.sum())

    # ---- per-edge slot (tile column, partition) + int32 gather index
    ecore = owner[dst_all]
    etile = tile_of[dst_all]
    ep = pslot[dst_all]
    o = np.argsort(dst_all, kind="stable")
    sd = dst_all[o]
    grp_start = np.r_[0, np.flatnonzero(np.diff(sd)) + 1]
    sizes = np.diff(np.r_[grp_start, sd.size])
    j_sorted = np.arange(sd.size) - np.repeat(grp_start, sizes)
    j = np.empty(sd.size, dtype=np.int64)
    j[o] = j_sorted

    col = col_base[etile] + j
    # gather row of the source in the all-gathered u (p-major shard dumps)
    pos = owner[src_all] * NSH + pslot[src_all] * T + tile_of[src_all]

    PAD_IDX = NSH - 1  # core 0's last dead row: always zero
    idx_vals = np.full((C, P, COLS), PAD_IDX, dtype=np.int32)
    idx_vals[ecore, ep, col] = pos.astype(np.int32)

    # ---- chunks of whole tiles (<= chunk_cols) with equal-J runs
    chunks = []  # (col_start, ncols, runs=[(t0, t1, J, col_off)])
    t = 0
    while t < T:
        t0 = t
        ncc = 0
        while t < T:
            step = int(Jt[t])
            if ncc > 0 and ncc + step > cfg.chunk_cols:
                break
            ncc += step
            t += 1
        runs = []
        rt = t0
        while rt < t:
            J = int(Jt[rt])
            rt1 = rt
            while rt1 < t and Jt[rt1] == J and rt1 - rt < cfg.max_run_tiles:
                rt1 += 1
            runs.append((rt, rt1, J, int(col_base[rt] - col_base[t0])))
            rt = rt1
        chunks.append((int(col_base[t0]), ncc, runs))

    # ---- per-core scale tiles [128, T] (dead rows -> 0)
    s2 = np.zeros((C, P, T), dtype=np.float32)   # 0.9*dinv^2
    d1 = np.zeros((C, P, T), dtype=np.float32)   # 0.9*dinv
    dv = np.zeros((C, P, T), dtype=np.float32)   # dinv
    s2[owner, pslot, tile_of] = (0.9 * dinv * dinv).astype(np.float32)
    d1[owner, pslot, tile_of] = (0.9 * dinv).astype(np.float32)
    dv[owner, pslot, tile_of] = dinv.astype(np.float32)

    sched = dict(chunks=chunks, COLS=COLS, Jt=Jt,
                 max_cols=max(ch[1] for ch in chunks))
    percore = dict(idx=idx_vals, s2=s2, d1=d1, dv=dv,
                   owner=owner, rank=rank, tile_of=tile_of, pslot=pslot)
    return sched, percore


# ---------------------------------------------------------------- device
def build_graph(sched, cfg, debug=False, reps=1):
    import concourse.bacc as bacc
    import concourse.bass as bass
    import concourse.tile as tile
    from concourse import mybir

    N, C, T, NSH, F = cfg.N, cfg.cores, cfg.T, cfg.NSH, cfg.F
    IN, HID = cfg.IN, cfg.HID
    COLS = sched["COLS"]
    chunks = sched["chunks"]
    max_cols = sched["max_cols"]
    FD = T * F
    f32 = mybir.dt.float32
    AX = mybir.AxisListType.X
    OP = mybir.AluOpType
    ACT = mybir.ActivationFunctionType

    nc = bacc.Bacc("TRN2", target_bir_lowering=False, debug=debug,
                   num_devices=C)

    xT_in = nc.dram_tensor("xT", [IN, NSH], f32, kind="ExternalInput")
    idx_in = nc.dram_tensor("idx", [P, COLS], mybir.dt.int32,
                            kind="ExternalInput")
    W1_in = nc.dram_tensor("W1", [IN, HID], f32, kind="ExternalInput")
    W2_in = nc.dram_tensor("W2", [HID, F], f32, kind="ExternalInput")
    b1_in = nc.dram_tensor("b1", [HID, 1], f32, kind="ExternalInput")
    b2r_in = nc.dram_tensor("b2r", [P, F], f32, kind="ExternalInput")
    s2_in = nc.dram_tensor("s2", [P, T], f32, kind="ExternalInput")
    d1_in = nc.dram_tensor("d1", [P, T], f32, kind="ExternalInput")
    dv_in = nc.dram_tensor("dv", [P, T], f32, kind="ExternalInput")
    out_ext = nc.dram_tensor("out", [P, FD], f32, kind="ExternalOutput")

    KB = IN // P   # k blocks (2)
    HB = HID // P  # h blocks (2)

    with tile.TileContext(nc) as tc:
        with (
            tc.tile_pool(name="statics", bufs=1) as statics,
            tc.tile_pool(name="resid", bufs=1) as resid,
            tc.tile_pool(name="xtp", bufs=1) as xtp,
            tc.tile_pool(name="mlp", bufs=4) as mlpp,
            tc.tile_pool(name="psum", bufs=3, space="PSUM") as psum,
            tc.tile_pool(name="msg", bufs=2) as msgp,
            tc.tile_pool(name="agg", bufs=2) as aggp,
            tc.tile_pool(name="unew", bufs=2) as unewp,
            tc.tile_pool(name="dram", bufs=1, space="DRAM") as dramp,
            tc.tile_pool(name="dram_stage", bufs=2, space="DRAM") as stagep,
            tc.tile_pool(name="dram_full", bufs=2, space="DRAM") as fullp,
        ):
            # ------- static loads
            idx_sb = statics.tile([P, COLS], mybir.dt.int32)
            nc.sync.dma_start(idx_sb[:], idx_in[:, :])
            w1_sb = statics.tile([P, KB * HID], f32)
            for kb in range(KB):
                nc.sync.dma_start(w1_sb[:, kb * HID:(kb + 1) * HID],
                                  W1_in[kb * P:(kb + 1) * P, :])
            w2_sb = statics.tile([P, HB * F], f32)
            for hb in range(HB):
                nc.sync.dma_start(w2_sb[:, hb * F:(hb + 1) * F],
                                  W2_in[hb * P:(hb + 1) * P, :])
            b1_sb = statics.tile([P, HB], f32)
            for hb in range(HB):
                nc.sync.dma_start(b1_sb[:, hb:hb + 1],
                                  b1_in[hb * P:(hb + 1) * P, :])
            b2r_sb = statics.tile([P, F], f32)
            nc.sync.dma_start(b2r_sb[:], b2r_in[:, :])
            s2_sb = statics.tile([P, T], f32)
            nc.sync.dma_start(s2_sb[:], s2_in[:, :])
            d1_sb = statics.tile([P, T], f32)
            nc.sync.dma_start(d1_sb[:], d1_in[:, :])
            dv_sb = statics.tile([P, T], f32)
            nc.sync.dma_start(dv_sb[:], dv_in[:, :])

            def bcast(t2d):  # [128, T] -> [128, T, F] broadcast AP
                return t2d[:, :, None].to_broadcast([P, T, F])

            for _rep in range(reps):
                # ------- MLP: h0 = relu(x@W1 + b1) @ W2 + b2 (n on ranks)
                h0_full = aggp.tile([P, FD], f32, tag="agg")
                halves = 4
                TH = (T + halves - 1) // halves
                for hv in range(halves):
                    t0 = hv * TH
                    t1 = min(t0 + TH, T)
                    if t0 >= t1:
                        continue
                    ncols = (t1 - t0) * P
                    xT_sb = xtp.tile([P, KB * TH * P], f32, tag="xt")
                    for kb in range(KB):
                        nc.sync.dma_start(
                            xT_sb[:, kb * ncols:(kb + 1) * ncols],
                            xT_in[kb * P:(kb + 1) * P, t0 * P:t1 * P])
                    for t in range(t0, t1):
                        c0 = (t - t0) * P
                        h1T = []
                        for hb in range(HB):
                            ps1 = psum.tile([P, P], f32, tag="ps1")
                            for kb in range(KB):
                                nc.tensor.matmul(
                                    ps1[:],
                                    lhsT=w1_sb[:, kb * HID + hb * P:
                                               kb * HID + (hb + 1) * P],
                                    rhs=xT_sb[:, kb * ncols + c0:
                                              kb * ncols + c0 + P],
                                    start=(kb == 0), stop=(kb == KB - 1))
                            h1_sb = mlpp.tile([P, P], f32, tag="h1")
                            nc.scalar.activation(h1_sb[:], ps1[:], ACT.Relu,
                                                 bias=b1_sb[:, hb:hb + 1])
                            h1T.append(h1_sb)
                        ps2 = psum.tile([P, F], f32, tag="ps2")
                        for hb in range(HB):
                            nc.tensor.matmul(
                                ps2[:], lhsT=h1T[hb][:],
                                rhs=w2_sb[:, hb * F:(hb + 1) * F],
                                start=(hb == 0), stop=(hb == HB - 1))
                        nc.vector.tensor_tensor(
                            out=h0_full[:, t * F:(t + 1) * F],
                            in0=ps2[:], in1=b2r_sb[:], op=OP.add)

                # u0 = dinv*h0 ; u0s = 0.1*u0 ; h0s = 0.1*h0
                u_sb = unewp.tile([P, FD], f32, tag="unew")
                nc.vector.tensor_tensor(
                    out=u_sb[:].rearrange("p (t f) -> p t f", f=F),
                    in0=h0_full[:].rearrange("p (t f) -> p t f", f=F),
                    in1=bcast(dv_sb), op=OP.mult)
                u0s_sb = resid.tile([P, FD], f32, tag="u0s")
                nc.vector.tensor_scalar_mul(u0s_sb[:], u_sb[:], ALPHA)
                h0s_dram = dramp.tile([P, FD], f32, tag="h0s")
                nc.scalar.mul(h0_full[:], h0_full[:], ALPHA)
                nc.sync.dma_start(h0s_dram[:], h0_full[:])

                u_stage = stagep.tile([P, FD], f32, tag="stage")
                nc.sync.dma_start(u_stage[:], u_sb[:])

                # ------- propagation hops
                import concourse.bass as bass_mod
                for k in range(cfg.hops):
                    u_full = fullp.tile([C * NSH, F], f32, tag="ufull")
                    nc.gpsimd.collective_compute(
                        "AllGather", OP.bypass,
                        replica_groups=[list(range(C))],
                        ins=[u_stage[:].opt()], outs=[u_full[:].opt()])

                    agg = aggp.tile([P, FD], f32, tag="agg")
                    for (col_start, ncc, runs) in chunks:
                        msg = msgp.tile([P, max_cols * F], f32, tag="msg")
                        for c in range(ncc):
                            nc.gpsimd.indirect_dma_start(
                                out=msg[:, c * F:(c + 1) * F],
                                out_offset=None,
                                in_=u_full[:, :],
                                in_offset=bass_mod.IndirectOffsetOnAxis(
                                    ap=idx_sb[:, col_start + c:
                                              col_start + c + 1],
                                    axis=0))
                        for (rt0, rt1, J, coff) in runs:
                            nt = rt1 - rt0
                            view = msg[:, coff * F:(coff + nt * J) * F] \
                                .rearrange("p (t j f) -> p t f j", j=J, f=F)
                            nc.vector.tensor_reduce(
                                out=agg[:, rt0 * F:rt1 * F].rearrange(
                                    "p (t f) -> p t f", f=F),
                                in_=view, axis=AX, op=OP.add)

                    if k < cfg.hops - 1:
                        u_new = unewp.tile([P, FD], f32, tag="unew")
                        nc.vector.tensor_tensor(
                            out=u_new[:].rearrange("p (t f) -> p t f", f=F),
                            in0=agg[:].rearrange("p (t f) -> p t f", f=F),
                            in1=bcast(s2_sb), op=OP.mult)
                        nc.vector.tensor_tensor(out=u_new[:], in0=u_new[:],
                                                in1=u0s_sb[:], op=OP.add)
                        u_stage = stagep.tile([P, FD], f32, tag="stage")
                        nc.sync.dma_start(u_stage[:], u_new[:])
                    else:
                        h_sb = unewp.tile([P, FD], f32, tag="unew")
                        nc.vector.tensor_tensor(
                            out=h_sb[:].rearrange("p (t f) -> p t f", f=F),
                            in0=agg[:].rearrange("p (t f) -> p t f", f=F),
                            in1=bcast(d1_sb), op=OP.mult)
                        h0s_sb = aggp.tile([P, FD], f32, tag="agg")
                        nc.sync.dma_start(h0s_sb[:], h0s_dram[:])
                        nc.vector.tensor_tensor(out=h_sb[:], in0=h_sb[:],
                                                in1=h0s_sb[:], op=OP.add)
                        # log_softmax over F
                        m_sb = mlpp.tile([P, T], f32, tag="lsm_m")
                        nc.vector.tensor_reduce(
                            out=m_sb[:],
                            in_=h_sb[:].rearrange("p (t f) -> p t f", f=F),
                            axis=AX, op=OP.max)
                        nc.vector.tensor_tensor(
                            out=h_sb[:].rearrange("p (t f) -> p t f", f=F),
                            in0=h_sb[:].rearrange("p (t f) -> p t f", f=F),
                            in1=bcast(m_sb), op=OP.subtract)
                        ex_sb = aggp.tile([P, FD], f32, tag="agg")
                        nc.scalar.activation(ex_sb[:], h_sb[:], ACT.Exp)
                        ssum = mlpp.tile([P, T], f32, tag="lsm_s")
                        nc.vector.tensor_reduce(
                            out=ssum[:],
                            in_=ex_sb[:].rearrange("p (t f) -> p t f", f=F),
                            axis=AX, op=OP.add)
                        lse = mlpp.tile([P, T], f32, tag="lsm_l")
                        nc.scalar.activation(lse[:], ssum[:], ACT.Ln)
                        nc.vector.tensor_tensor(
                            out=h_sb[:].rearrange("p (t f) -> p t f", f=F),
                            in0=h_sb[:].rearrange("p (t f) -> p t f", f=F),
                            in1=bcast(lse), op=OP.subtract)
                        nc.sync.dma_start(out_ext[:, :], h_sb[:])

    nc.compile()
    return nc


# ---------------------------------------------------------------- top level
def _make_inputs(x, W1, b1, W2, b2, sched, percore, cfg):
    C, NSH = cfg.cores, cfg.NSH
    owner, rank = percore["owner"], percore["rank"]
    in_maps = []
    x = np.asarray(x, dtype=np.float32)
    W1 = np.asarray(W1, dtype=np.float32)
    W2 = np.asarray(W2, dtype=np.float32)
    b1 = np.asarray(b1, dtype=np.float32).reshape(-1, 1)
    b2r = np.tile(np.asarray(b2, dtype=np.float32).reshape(1, -1), (P, 1))
    for c in range(C):
        nodes = np.flatnonzero(owner == c)
        r = rank[nodes]
        xT = np.zeros((cfg.IN, NSH), dtype=np.float32)
        xT[:, r] = x[nodes].T
        in_maps.append(dict(
            xT=xT, idx=percore["idx"][c],
            W1=W1, W2=W2, b1=b1, b2r=b2r,
            s2=percore["s2"][c], d1=percore["d1"][c], dv=percore["dv"][c],
        ))
    return in_maps


def _unshard_output(results, percore, cfg):
    N, C, T, F = cfg.N, cfg.cores, cfg.T, cfg.F
    owner, pslot, tile_of = (percore["owner"], percore["pslot"],
                             percore["tile_of"])
    full = np.empty((N, F), dtype=np.float32)
    for c in range(C):
        oc = results[c]["out"].reshape(P, T, F)
        nodes = np.flatnonzero(owner == c)
        full[nodes] = oc[pslot[nodes], tile_of[nodes], :]
    return full


def run(x, edge_index, W1, b1, W2, b2, cfg=FULL, reps=1, nc=None,
        prep=None):
    from concourse.bass_utils import run_bass_kernel_spmd

    N = cfg.N
    if prep is None:
        dst = np.asarray(edge_index[1], dtype=np.int64)
        deg = np.bincount(np.concatenate([dst, np.arange(N)]),
                          minlength=N).astype(np.float64)
        dinv = 1.0 / np.sqrt(deg)
        prep = preprocess(edge_index, dinv, cfg)
    sched, percore = prep
    if nc is None:
        nc = build_graph(sched, cfg, reps=reps)
    in_maps = _make_inputs(x, W1, b1, W2, b2, sched, percore, cfg)
    res = run_bass_kernel_spmd(nc, in_maps, core_ids=list(range(cfg.cores)))
    out = _unshard_output(res.results, percore, cfg)
    return out, res, nc, prep


def kernel(x, edge_index, W1, b1, W2, b2):
    out, _, _, _ = run(x, edge_index, W1, b1, W2, b2, FULL)
    return out



# revision 2
# speedup vs baseline: 4.7059x; 4.7059x over previous
"""APPNP (MLP + K-step personalized-PageRank propagation) on 8 TRN2 NeuronCores.

Strategy
--------
* Nodes are sharded across the 8 cores (12500 + 44 dead pad rows each).
* norm = dinv[src]*dinv[dst] factorizes, so each hop is:
      agg = A^T u  with  u = dinv*h  (gather u[src] per edge + segment-sum)
      u' = (0.9*dinv^2)*agg + 0.1*dinv*h0
* Per hop: AllGather of per-core u shards (bf16) -> full u [100352, 64] in
  each core's DRAM, then indexed-row DMA gathers (gpsimd indirect_dma_start,
  one 128-row descriptor batch per call) pull per-edge source rows into
  SBUF in a static ELL layout (dst-tile x neighbor-slot grid, zero-row
  pads), and strided DVE tensor_reduce does the segment-sum (f32 accum).
* The ELL grid: per-core dsts are degree-sorted into 98 tiles of 128;
  tile t gets J[t] = max in-degree columns. Gather call (t, j) fetches,
  for all 128 dsts p of tile t, the row of their j-th in-neighbor
  (int32 indices, per-call index column [128, 1]).
* Transfer-optimized I/O (the axon tunnel is the wall-clock bottleneck):
  x ships as fp8-e4m3 (PE does bf16xfp8 matmul directly), W1/W2 as bf16,
  the output as bf16; only dinv ships for the scales (0.9*dinv and
  0.9*dinv^2 are derived on device). End-to-end rel err ~4e-3 vs the
  2e-2 gate (validated against the fp32 reference).
* All static structure is computed on the host from edge_index inside
  kernel(); the 8 cores run one SPMD graph with identical shapes; a
  pre-jitted shard_map executor is reused across calls (no per-call
  retrace), with inputs device_put fresh on every call.
"""

import math
import sys
import numpy as np

try:  # concourse ships in the container; add its repo root if not on path
    import concourse  # noqa: F401
except ImportError:  # pragma: no cover
    for _p in ("/root/.axon_site/_ro/trn_rl_repo", "/opt/trn_rl_repo"):
        if _p not in sys.path:
            sys.path.insert(0, _p)
    import concourse  # noqa: F401

# ---------------------------------------------------------------- constants
K_HOPS = 10
ALPHA = 0.1
P = 128  # partitions


class Cfg:
    def __init__(self, N=100000, E=1600000, IN=256, HID=256, OUT=64, cores=8,
                 chunk_cols=48, max_run_tiles=16, hops=K_HOPS):
        self.N, self.E, self.IN, self.HID, self.OUT = N, E, IN, HID, OUT
        self.cores = cores
        self.per = N // cores                       # real nodes per core
        self.T = math.ceil(self.per / P)            # tiles per core
        if self.T * P == self.per:
            self.T += 1  # need >=1 dead (always-zero) row for gather pads
        self.NSH = self.T * P                       # padded rows per core
        self.chunk_cols = chunk_cols                # gather cols per msg tile
        self.max_run_tiles = max_run_tiles          # cap tiles per reduce op
        self.hops = hops
        self.F = self.OUT


FULL = Cfg()


# ---------------------------------------------------------------- host prep
def preprocess(edge_index, dinv, cfg, seed=0):
    """Static schedule + per-core input arrays.

    sched: shape-defining info shared by all cores (ELL tile degrees J,
    chunk/run grouping).  percore: per-core index & scale tensors.
    """
    N, C, T, NSH, F = cfg.N, cfg.cores, cfg.T, cfg.NSH, cfg.F
    per = cfg.per

    src = np.asarray(edge_index[0], dtype=np.int64)
    dst = np.asarray(edge_index[1], dtype=np.int64)
    loops = np.arange(N, dtype=np.int64)
    src_all = np.concatenate([src, loops])
    dst_all = np.concatenate([dst, loops])
    deg = np.bincount(dst_all, minlength=N)

    # ---- node -> core assignment (random balanced)
    rng = np.random.default_rng(seed)
    perm = rng.permutation(N)
    owner = np.empty(N, dtype=np.int64)
    for c in range(C):
        owner[perm[c * per:(c + 1) * per]] = c

    # ---- per-core rank: in-degree descending -> tight per-tile max degree
    rank = np.empty(N, dtype=np.int64)
    for c in range(C):
        nodes = perm[c * per:(c + 1) * per]
        order = np.argsort(-deg[nodes], kind="stable")
        rank[nodes[order]] = np.arange(per)
    tile_of = rank // P
    pslot = rank % P

    # ---- uniform-across-cores tile max degree
    Jt = np.zeros(T, dtype=np.int64)
    np.maximum.at(Jt, tile_of, deg)
    Jt[:] = np.maximum(Jt, 1)

    col_base = np.zeros(T, dtype=np.int64)
    col_base[1:] = np.cumsum(Jt)[:-1]
    COLS = int(Jt.sum())

    # ---- per-edge slot (tile column, partition) + int32 gather index
    ecore = owner[dst_all]
    etile = tile_of[dst_all]
    ep = pslot[dst_all]
    o = np.argsort(dst_all, kind="stable")
    sd = dst_all[o]
    grp_start = np.r_[0, np.flatnonzero(np.diff(sd)) + 1]
    sizes = np.diff(np.r_[grp_start, sd.size])
    j_sorted = np.arange(sd.size) - np.repeat(grp_start, sizes)
    j = np.empty(sd.size, dtype=np.int64)
    j[o] = j_sorted

    col = col_base[etile] + j
    # gather row of the source in the all-gathered u (p-major shard dumps)
    pos = owner[src_all] * NSH + pslot[src_all] * T + tile_of[src_all]

    PAD_IDX = NSH - 1  # core 0's last dead row: always zero
    idx_vals = np.full((C, P, COLS), PAD_IDX, dtype=np.int32)
    idx_vals[ecore, ep, col] = pos.astype(np.int32)

    # ---- chunks of whole tiles (<= chunk_cols) with equal-J runs
    chunks = []  # (col_start, ncols, runs=[(t0, t1, J, col_off)])
    t = 0
    while t < T:
        t0 = t
        ncc = 0
        while t < T:
            step = int(Jt[t])
            if ncc > 0 and ncc + step > cfg.chunk_cols:
                break
            ncc += step
            t += 1
        runs = []
        rt = t0
        while rt < t:
            J = int(Jt[rt])
            rt1 = rt
            while rt1 < t and Jt[rt1] == J and rt1 - rt < cfg.max_run_tiles:
                rt1 += 1
            runs.append((rt, rt1, J, int(col_base[rt] - col_base[t0])))
            rt = rt1
        chunks.append((int(col_base[t0]), ncc, runs))

    # ---- per-core dinv tile [128, T] (dead rows -> 0); 0.9*dinv and
    # 0.9*dinv^2 are derived on device
    dv = np.zeros((C, P, T), dtype=np.float32)
    dv[owner, pslot, tile_of] = dinv.astype(np.float32)

    sched = dict(chunks=chunks, COLS=COLS, Jt=Jt,
                 max_cols=max(ch[1] for ch in chunks))
    percore = dict(idx=idx_vals, dv=dv,
                   owner=owner, rank=rank, tile_of=tile_of, pslot=pslot)
    return sched, percore


# ---------------------------------------------------------------- device
def build_graph(sched, cfg, debug=False, reps=1):
    import concourse.bacc as bacc
    import concourse.bass as bass_mod
    import concourse.tile as tile
    from concourse import mybir

    N, C, T, NSH, F = cfg.N, cfg.cores, cfg.T, cfg.NSH, cfg.F
    IN, HID = cfg.IN, cfg.HID
    COLS = sched["COLS"]
    chunks = sched["chunks"]
    max_cols = sched["max_cols"]
    FD = T * F
    f32 = mybir.dt.float32
    bf16 = mybir.dt.bfloat16
    f8 = mybir.dt.float8e4
    AX = mybir.AxisListType.X
    OP = mybir.AluOpType
    ACT = mybir.ActivationFunctionType

    nc = bacc.Bacc("TRN2", target_bir_lowering=False, debug=debug,
                   num_devices=C)

    xT_in = nc.dram_tensor("xT", [IN, NSH], f8, kind="ExternalInput")
    idx_in = nc.dram_tensor("idx", [P, COLS], mybir.dt.int32,
                            kind="ExternalInput")
    W1_in = nc.dram_tensor("W1", [IN, HID], bf16, kind="ExternalInput")
    W2_in = nc.dram_tensor("W2", [HID, F], bf16, kind="ExternalInput")
    b1_in = nc.dram_tensor("b1", [HID, 1], f32, kind="ExternalInput")
    b2r_in = nc.dram_tensor("b2r", [P, F], f32, kind="ExternalInput")
    dv_in = nc.dram_tensor("dv", [P, T], f32, kind="ExternalInput")
    out_ext = nc.dram_tensor("out", [P, FD], bf16, kind="ExternalOutput")

    KB = IN // P   # k blocks (2)
    HB = HID // P  # h blocks (2)

    with tile.TileContext(nc) as tc:
        with (
            tc.tile_pool(name="statics", bufs=1) as statics,
            tc.tile_pool(name="resid", bufs=1) as resid,
            tc.tile_pool(name="xtp", bufs=1) as xtp,
            tc.tile_pool(name="mlp", bufs=4) as mlpp,
            tc.tile_pool(name="psum", bufs=3, space="PSUM") as psum,
            tc.tile_pool(name="msg", bufs=2) as msgp,
            tc.tile_pool(name="agg", bufs=2) as aggp,
            tc.tile_pool(name="unew", bufs=2) as unewp,
            tc.tile_pool(name="ubf", bufs=2) as ubfp,
            tc.tile_pool(name="dram", bufs=1, space="DRAM") as dramp,
            tc.tile_pool(name="dram_stage", bufs=2, space="DRAM") as stagep,
            tc.tile_pool(name="dram_full", bufs=2, space="DRAM") as fullp,
        ):
            # ------- static loads
            idx_sb = statics.tile([P, COLS], mybir.dt.int32)
            nc.sync.dma_start(idx_sb[:], idx_in[:, :])
            w1_sb = statics.tile([P, KB * HID], bf16)
            for kb in range(KB):
                nc.sync.dma_start(w1_sb[:, kb * HID:(kb + 1) * HID],
                                  W1_in[kb * P:(kb + 1) * P, :])
            w2_sb = statics.tile([P, HB * F], bf16)
            for hb in range(HB):
                nc.sync.dma_start(w2_sb[:, hb * F:(hb + 1) * F],
                                  W2_in[hb * P:(hb + 1) * P, :])
            b1_sb = statics.tile([P, HB], f32)
            for hb in range(HB):
                nc.sync.dma_start(b1_sb[:, hb:hb + 1],
                                  b1_in[hb * P:(hb + 1) * P, :])
            b2r_sb = statics.tile([P, F], f32)
            nc.sync.dma_start(b2r_sb[:], b2r_in[:, :])
            dv_sb = statics.tile([P, T], f32)
            nc.sync.dma_start(dv_sb[:], dv_in[:, :])
            # derived scales: s2 = 0.9*dinv^2, d1 = 0.9*dinv
            s2_sb = statics.tile([P, T], f32)
            nc.vector.tensor_tensor(out=s2_sb[:], in0=dv_sb[:], in1=dv_sb[:],
                                    op=OP.mult)
            nc.vector.tensor_scalar_mul(s2_sb[:], s2_sb[:], 1.0 - ALPHA)
            d1_sb = statics.tile([P, T], f32)
            nc.vector.tensor_scalar_mul(d1_sb[:], dv_sb[:], 1.0 - ALPHA)

            def bcast(t2d):  # [128, T] -> [128, T, F] broadcast AP
                return t2d[:, :, None].to_broadcast([P, T, F])

            for _rep in range(reps):
                # ------- MLP: h0 = relu(x@W1 + b1) @ W2 + b2 (n on ranks)
                h0_full = aggp.tile([P, FD], f32, tag="agg")
                halves = 4
                TH = (T + halves - 1) // halves
                for hv in range(halves):
                    t0 = hv * TH
                    t1 = min(t0 + TH, T)
                    if t0 >= t1:
                        continue
                    ncols = (t1 - t0) * P
                    xT_sb = xtp.tile([P, KB * TH * P], f8, tag="xt")
                    for kb in range(KB):
                        nc.sync.dma_start(
                            xT_sb[:, kb * ncols:(kb + 1) * ncols],
                            xT_in[kb * P:(kb + 1) * P, t0 * P:t1 * P])
                    for t in range(t0, t1):
                        c0 = (t - t0) * P
                        h1T = []
                        for hb in range(HB):
                            ps1 = psum.tile([P, P], f32, tag="ps1")
                            for kb in range(KB):
                                nc.tensor.matmul(
                                    ps1[:],
                                    lhsT=w1_sb[:, kb * HID + hb * P:
                                               kb * HID + (hb + 1) * P],
                                    rhs=xT_sb[:, kb * ncols + c0:
                                              kb * ncols + c0 + P],
                                    start=(kb == 0), stop=(kb == KB - 1))
                            h1_sb = mlpp.tile([P, P], bf16, tag="h1")
                            nc.scalar.activation(h1_sb[:], ps1[:], ACT.Relu,
                                                 bias=b1_sb[:, hb:hb + 1])
                            h1T.append(h1_sb)
                        ps2 = psum.tile([P, F], f32, tag="ps2")
                        for hb in range(HB):
                            nc.tensor.matmul(
                                ps2[:], lhsT=h1T[hb][:],
                                rhs=w2_sb[:, hb * F:(hb + 1) * F],
                                start=(hb == 0), stop=(hb == HB - 1))
                        nc.vector.tensor_tensor(
                            out=h0_full[:, t * F:(t + 1) * F],
                            in0=ps2[:], in1=b2r_sb[:], op=OP.add)

                # u0 = dinv*h0 ; u0s = 0.1*u0 ; h0s = 0.1*h0
                u_sb = unewp.tile([P, FD], f32, tag="unew")
                nc.vector.tensor_tensor(
                    out=u_sb[:].rearrange("p (t f) -> p t f", f=F),
                    in0=h0_full[:].rearrange("p (t f) -> p t f", f=F),
                    in1=bcast(dv_sb), op=OP.mult)
                u0s_sb = resid.tile([P, FD], f32, tag="u0s")
                nc.vector.tensor_scalar_mul(u0s_sb[:], u_sb[:], ALPHA)
                h0s_dram = dramp.tile([P, FD], f32, tag="h0s")
                nc.scalar.mul(h0_full[:], h0_full[:], ALPHA)
                nc.sync.dma_start(h0s_dram[:], h0_full[:])

                u_bf = ubfp.tile([P, FD], bf16, tag="ubf")
                nc.vector.tensor_copy(u_bf[:], u_sb[:])
                u_stage = stagep.tile([P, FD], bf16, tag="stage")
                nc.sync.dma_start(u_stage[:], u_bf[:])

                # ------- propagation hops
                for k in range(cfg.hops):
                    u_full = fullp.tile([C * NSH, F], bf16, tag="ufull")
                    nc.gpsimd.collective_compute(
                        "AllGather", OP.bypass,
                        replica_groups=[list(range(C))],
                        ins=[u_stage[:].opt()], outs=[u_full[:].opt()])

                    agg = aggp.tile([P, FD], f32, tag="agg")
                    for (col_start, ncc, runs) in chunks:
                        msg = msgp.tile([P, max_cols * F], bf16, tag="msg")
                        for c in range(ncc):
                            nc.gpsimd.indirect_dma_start(
                                out=msg[:, c * F:(c + 1) * F],
                                out_offset=None,
                                in_=u_full[:, :],
                                in_offset=bass_mod.IndirectOffsetOnAxis(
                                    ap=idx_sb[:, col_start + c:
                                              col_start + c + 1],
                                    axis=0))
                        for (rt0, rt1, J, coff) in runs:
                            nt = rt1 - rt0
                            view = msg[:, coff * F:(coff + nt * J) * F] \
                                .rearrange("p (t j f) -> p t f j", j=J, f=F)
                            nc.vector.tensor_reduce(
                                out=agg[:, rt0 * F:rt1 * F].rearrange(
                                    "p (t f) -> p t f", f=F),
                                in_=view, axis=AX, op=OP.add)

                    if k < cfg.hops - 1:
                        u_new = unewp.tile([P, FD], f32, tag="unew")
                        nc.vector.tensor_tensor(
                            out=u_new[:].rearrange("p (t f) -> p t f", f=F),
                            in0=agg[:].rearrange("p (t f) -> p t f", f=F),
                            in1=bcast(s2_sb), op=OP.mult)
                        u_bf = ubfp.tile([P, FD], bf16, tag="ubf")
                        nc.vector.tensor_tensor(out=u_bf[:], in0=u_new[:],
                                                in1=u0s_sb[:], op=OP.add)
                        u_stage = stagep.tile([P, FD], bf16, tag="stage")
                        nc.sync.dma_start(u_stage[:], u_bf[:])
                    else:
                        h_sb = unewp.tile([P, FD], f32, tag="unew")
                        nc.vector.tensor_tensor(
                            out=h_sb[:].rearrange("p (t f) -> p t f", f=F),
                            in0=agg[:].rearrange("p (t f) -> p t f", f=F),
                            in1=bcast(d1_sb), op=OP.mult)
                        h0s_sb = aggp.tile([P, FD], f32, tag="agg")
                        nc.sync.dma_start(h0s_sb[:], h0s_dram[:])
                        nc.vector.tensor_tensor(out=h_sb[:], in0=h_sb[:],
                                                in1=h0s_sb[:], op=OP.add)
                        # log_softmax over F
                        m_sb = mlpp.tile([P, T], f32, tag="lsm_m")
                        nc.vector.tensor_reduce(
                            out=m_sb[:],
                            in_=h_sb[:].rearrange("p (t f) -> p t f", f=F),
                            axis=AX, op=OP.max)
                        nc.vector.tensor_tensor(
                            out=h_sb[:].rearrange("p (t f) -> p t f", f=F),
                            in0=h_sb[:].rearrange("p (t f) -> p t f", f=F),
                            in1=bcast(m_sb), op=OP.subtract)
                        ex_sb = aggp.tile([P, FD], f32, tag="agg")
                        nc.scalar.activation(ex_sb[:], h_sb[:], ACT.Exp)
                        ssum = mlpp.tile([P, T], f32, tag="lsm_s")
                        nc.vector.tensor_reduce(
                            out=ssum[:],
                            in_=ex_sb[:].rearrange("p (t f) -> p t f", f=F),
                            axis=AX, op=OP.add)
                        lse = mlpp.tile([P, T], f32, tag="lsm_l")
                        nc.scalar.activation(lse[:], ssum[:], ACT.Ln)
                        o_bf = ubfp.tile([P, FD], bf16, tag="ubf")
                        nc.vector.tensor_tensor(
                            out=o_bf[:].rearrange("p (t f) -> p t f", f=F),
                            in0=h_sb[:].rearrange("p (t f) -> p t f", f=F),
                            in1=bcast(lse), op=OP.subtract)
                        nc.sync.dma_start(out_ext[:, :], o_bf[:])

    nc.compile()
    return nc


# ---------------------------------------------------------------- executor
class Executor:
    """Pre-jitted SPMD runner: one shard_map jit built at construction,
    reused for every call. Each call device_puts the (host) global input
    arrays fresh, executes, and materializes host outputs — i.e. the full
    host->device->host round trip is inside __call__."""

    def __init__(self, nc, n_cores):
        import jax
        from concourse import mybir
        from concourse.bass2jax import (_bass_exec_p, install_neuronx_cc_hook,
                                        partition_id_tensor)
        from jax.sharding import Mesh, PartitionSpec, NamedSharding
        from jax.experimental.shard_map import shard_map

        install_neuronx_cc_hook()
        self.nc = nc
        self.n_cores = n_cores
        pname = nc.partition_id_tensor.name if nc.partition_id_tensor else None
        in_names, out_names, out_avals, zero_outs = [], [], [], []
        for alloc in nc.m.functions[0].allocations:
            if not isinstance(alloc, mybir.MemoryLocationSet):
                continue
            name = alloc.memorylocations[0].name
            if alloc.kind == "ExternalInput":
                if name != pname:
                    in_names.append(name)
            elif alloc.kind == "ExternalOutput":
                out_names.append(name)
                out_avals.append(jax.core.ShapedArray(
                    tuple(alloc.tensor_shape), mybir.dt.np(alloc.dtype)))
                zero_outs.append(np.zeros(tuple(alloc.tensor_shape),
                                          mybir.dt.np(alloc.dtype)))
        self.in_names, self.out_names = in_names, out_names
        n_params, n_outs = len(in_names), len(out_avals)
        in_names_full = in_names + out_names + ([pname] if pname else [])
        donate = tuple(range(n_params, n_params + n_outs))
        self._zero_glob = [
            np.zeros((n_cores * z.shape[0], *z.shape[1:]), z.dtype)
            for z in zero_outs]
        self._out_shapes = [a.shape for a in out_avals]

        def _body(*args):
            operands = list(args)
            if pname is not None:
                operands.append(partition_id_tensor())
            return tuple(_bass_exec_p.bind(
                *operands, out_avals=tuple(out_avals),
                in_names=tuple(in_names_full), out_names=tuple(out_names),
                lowering_input_output_aliases=(), sim_require_finite=True,
                sim_require_nnan=True, nc=nc))

        devices = jax.devices()[:n_cores]
        mesh = Mesh(np.asarray(devices), ("core",))
        self._sharding = NamedSharding(mesh, PartitionSpec("core"))
        in_specs = (PartitionSpec("core"),) * (n_params + n_outs)
        out_specs = (PartitionSpec("core"),) * n_outs
        self._fn = jax.jit(
            shard_map(_body, mesh=mesh, in_specs=in_specs,
                      out_specs=out_specs, check_rep=False),
            donate_argnums=donate, keep_unused=True)
        self._jax = jax

    def __call__(self, in_glob):
        """in_glob: list of global (n_cores*dim0, ...) np arrays ordered as
        self.in_names. Returns list of global np output arrays."""
        jax = self._jax
        dev_in = [jax.device_put(a, self._sharding) for a in in_glob]
        dev_z = [jax.device_put(z, self._sharding) for z in self._zero_glob]
        outs = self._fn(*dev_in, *dev_z)
        return [np.asarray(o) for o in outs]


# ---------------------------------------------------------------- top level
def _make_inputs(x, W1, b1, W2, b2, sched, percore, cfg, in_names):
    """Global (concatenated-over-cores) input arrays, ordered by in_names."""
    import ml_dtypes
    C, NSH = cfg.cores, cfg.NSH
    owner, rank = percore["owner"], percore["rank"]
    x = np.asarray(x, dtype=np.float32)
    W1b = np.asarray(W1, dtype=np.float32).astype(ml_dtypes.bfloat16)
    W2b = np.asarray(W2, dtype=np.float32).astype(ml_dtypes.bfloat16)
    b1c = np.asarray(b1, dtype=np.float32).reshape(-1, 1)
    b2r = np.tile(np.asarray(b2, dtype=np.float32).reshape(1, -1), (P, 1))
    per_core = []
    for c in range(C):
        nodes = np.flatnonzero(owner == c)
        r = rank[nodes]
        xT = np.zeros((cfg.IN, NSH), dtype=np.float32)
        xT[:, r] = x[nodes].T
        per_core.append(dict(
            xT=xT.astype(ml_dtypes.float8_e4m3), idx=percore["idx"][c],
            W1=W1b, W2=W2b, b1=b1c, b2r=b2r, dv=percore["dv"][c],
        ))
    return [np.concatenate([per_core[c][name] for c in range(C)], axis=0)
            for name in in_names]


def _unshard_output(out_glob, percore, cfg):
    N, C, T, F = cfg.N, cfg.cores, cfg.T, cfg.F
    owner, pslot, tile_of = (percore["owner"], percore["pslot"],
                             percore["tile_of"])
    full = np.empty((N, F), dtype=np.float32)
    per = out_glob.reshape(C, P, T, F)
    for c in range(C):
        nodes = np.flatnonzero(owner == c)
        full[nodes] = per[c][pslot[nodes], tile_of[nodes], :] \
            .astype(np.float32)
    return full


def run(x, edge_index, W1, b1, W2, b2, cfg=FULL, reps=1, nc=None,
        prep=None, ex=None):
    N = cfg.N
    if prep is None:
        dst = np.asarray(edge_index[1], dtype=np.int64)
        deg = np.bincount(np.concatenate([dst, np.arange(N)]),
                          minlength=N).astype(np.float64)
        dinv = 1.0 / np.sqrt(deg)
        prep = preprocess(edge_index, dinv, cfg)
    sched, percore = prep
    if nc is None:
        nc = build_graph(sched, cfg, reps=reps)
    if ex is None:
        ex = Executor(nc, cfg.cores)
    in_glob = _make_inputs(x, W1, b1, W2, b2, sched, percore, cfg,
                           ex.in_names)
    outs = ex(in_glob)
    out = _unshard_output(outs[ex.out_names.index("out")], percore, cfg)
    return out, ex, nc, prep, in_glob


def kernel(x, edge_index, W1, b1, W2, b2):
    out, _, _, _, _ = run(x, edge_index, W1, b1, W2, b2, FULL)
    return out
